# revision 13
# baseline (speedup 1.0000x reference)
"""Trainium2 Bass kernel for nn_CustomParameterTransform (scatter_memory).

Reference semantics: coord_v [256, 30] holds 10 (x, y, mass) triplets per
sample. Each triplet maps to integer grid indices (x_i, y_i, m_i); a one-hot
volume z [B, 16, 128, 128] is scattered (z[b, m, y, x] = 1) and the output is
concat(1-z, z) over the channel axis -> [256, 32, 128, 128] f32 (512 MB).

Strategy (8 NeuronCores, batch-sharded, no cross-core comm):
  - The output is almost entirely constant: the first 16 channels are 1.0
    except at scatter points, the last 16 are 0.0 except at scatter points.
  - Per core (32 samples, 64 MB slab): fill the slab from a constant SBUF
    tile with large DMAs (write-only HBM traffic, the memory roofline),
    then fix up the 640 scatter points with indirect (scatter) DMAs.
  - Indices are computed on the host with the exact same jax ops as the
    reference (bit-identical floor/log10 behavior) and passed per-core as a
    [128, 5] int32 tensor of flat element offsets.
"""

import numpy as np

B = 256
NSRC = 10
NMC = 16
L = 128
NCORES = 8
BL = B // NCORES          # 32 samples per core
PLANE = L * L             # 16384
HALF = NMC * PLANE        # 262144 elements per half-slab
SLAB = 2 * HALF           # 524288 elements per sample
OUT_ELEMS = BL * SLAB     # 16777216 per core (64 MB)

N_SCATTER_COLS = 5        # 640 scatter writes = 128 partitions x 5 columns

_CACHE = {}


def _build_nc():
    import concourse.bass as bass
    import concourse.tile as tile
    from concourse import bacc, mybir
    from concourse.tile_rust import add_dep_helper

    nc = bacc.Bacc("TRN2", target_bir_lowering=False, debug=False,
                   num_devices=NCORES)

    offs = nc.dram_tensor("offs", [128, N_SCATTER_COLS], mybir.dt.int32,
                          kind="ExternalInput").ap()
    out = nc.dram_tensor("out", [OUT_ELEMS], mybir.dt.float32,
                         kind="ExternalOutput").ap()

    with tile.TileContext(nc) as tc:
        with tc.tile_pool(name="src", bufs=1) as src_pool, \
             tc.tile_pool(name="small", bufs=1) as small_pool:
            # Constant source tiles. combo_t is one full 2 MB sample slab
            # ([128, 4096] f32): DMA iterates partition-major, so partitions
            # 0-63 are the ones half and 64-127 the zeros half. Its two
            # memsets take ~3.5 us each (parallel on two engines), so the
            # first two slabs are instead fed from 256 KB mini tiles whose
            # memsets take ~0.4 us — fills start ~5 us earlier.
            ones_mini = src_pool.tile([128, 512], mybir.dt.float32)
            zeros_mini = src_pool.tile([128, 512], mybir.dt.float32)
            nc.vector.memset(ones_mini[:, :], 1.0)
            nc.gpsimd.memset(zeros_mini[:, :], 0.0)
            combo_t = src_pool.tile([128, 4096], mybir.dt.float32)
            nc.vector.memset(combo_t[0:64, :], 1.0)
            nc.gpsimd.memset(combo_t[64:128, :], 0.0)

            # Scatter offsets: [128, 5] int32 flat element indices.
            # Column layout (entries are points p = 10*s + k, in order):
            #   col 0: ones-half offsets for points   0..127 (samples  0-12)
            #   col 1: z-half    offsets for points   0..127 (samples  0-12)
            #   col 2: ones-half offsets for points 128..255 (samples 12-25)
            #   col 3: z-half    offsets for points 128..255 (samples 12-25)
            #   col 4: rows 0-63 ones-half pts 256..319, rows 64-127 z-half
            #          pts 256..319 (samples 25-31)
            # offs load and vals memsets are only needed by the scatters
            # (earliest ~70 us in) — keep them off the fill rings / after
            # the mini-tile memsets so they don't delay the first fills.
            offs_t = small_pool.tile([128, N_SCATTER_COLS], mybir.dt.int32)
            nc.gpsimd.dma_start(offs_t[:, :], offs[:, :])
            vals_t = small_pool.tile([128, N_SCATTER_COLS], mybir.dt.float32)
            nc.gpsimd.memset(vals_t[:, 0:1], 0.0)
            nc.gpsimd.memset(vals_t[:, 1:2], 1.0)
            nc.gpsimd.memset(vals_t[:, 2:3], 0.0)
            nc.gpsimd.memset(vals_t[:, 3:4], 1.0)
            nc.gpsimd.memset(vals_t[0:64, 4:5], 0.0)
            nc.gpsimd.memset(vals_t[64:128, 4:5], 1.0)

            MINI = 65536  # elements per mini fill (256 KB)
            ones_fills = {}   # sample -> list of fills covering its ones half
            zeros_fills = {}  # sample -> list of fills covering its zeros half
            for s in (0, 1):
                e_ones = nc.sync if s == 0 else nc.scalar
                e_zeros = nc.scalar if s == 0 else nc.sync
                ones_fills[s] = [
                    e_ones.dma_start(
                        out[s * SLAB + k * MINI:s * SLAB + (k + 1) * MINI],
                        ones_mini[:, :])
                    for k in range(4)]
                zeros_fills[s] = [
                    e_zeros.dma_start(
                        out[s * SLAB + HALF + k * MINI:
                            s * SLAB + HALF + (k + 1) * MINI],
                        zeros_mini[:, :])
                    for k in range(4)]
            for s in range(2, BL):
                eng = nc.sync if s % 2 == 0 else nc.scalar
                f = eng.dma_start(out[s * SLAB:(s + 1) * SLAB], combo_t[:, :])
                ones_fills[s] = [f]
                zeros_fills[s] = [f]

            # Which sample-fills each scatter column touches.
            def deps(table, lo, hi):
                return [f for s in range(lo, hi) for f in table[s]]
            col_deps = [
                deps(ones_fills, 0, 13),
                deps(zeros_fills, 0, 13),
                deps(ones_fills, 12, 26),
                deps(zeros_fills, 12, 26),
                deps(ones_fills, 25, BL) + deps(zeros_fills, 25, BL),
            ]

            # Narrow declared out AP ([1, 1] at offset 0): the real write
            # addresses come from the offset tensor; a full-tensor AP would
            # make Tile serialize every scatter behind every fill (WAW), and
            # the explicit col_deps edges below provide the true ordering.
            out2d = out[0:1].unsqueeze(1)
            for j in range(N_SCATTER_COLS):
                sc = nc.gpsimd.indirect_dma_start(
                    out=out2d,
                    out_offset=bass.IndirectOffsetOnAxis(
                        ap=offs_t[:, j:j + 1], axis=0),
                    in_=vals_t[:, j:j + 1],
                    in_offset=None,
                )
                for fl in col_deps[j]:
                    add_dep_helper(sc.ins, fl.ins,
                                   reason="scatter after its sample fills")

    nc.compile()
    return nc


def _compute_indices(coord_v, lows, highs, nmc, L_):
    """Replicates reference.py lines exactly (same jax ops on the default
    device) so the floor/log10 bin boundaries match bit-for-bit."""
    import jax.numpy as jnp

    cv = jnp.asarray(np.asarray(coord_v, dtype=np.float32))
    n = cv.shape[1] // 3
    v10 = cv.at[:, 2::3].set(jnp.log10(cv[:, 2::3]))
    lo = jnp.tile(jnp.asarray(np.asarray(lows, dtype=np.float32)), n)
    hi = jnp.tile(jnp.asarray(np.asarray(highs, dtype=np.float32)), n)
    coord_grid = (v10 - lo) / (hi - lo)
    tr = coord_grid.reshape(-1, 3)
    x_i = jnp.floor(tr[:, 0] * L_).astype(jnp.int32)
    y_i = jnp.floor(tr[:, 1] * L_).astype(jnp.int32)
    m_i = jnp.floor(tr[:, 2] * nmc).astype(jnp.int32)
    return (np.asarray(x_i), np.asarray(y_i), np.asarray(m_i))


def _prepare_in_maps(coord_v, lows, highs, nmc, L):
    nmc = int(nmc)
    L_ = int(L)
    x_i, y_i, m_i = _compute_indices(coord_v, lows, highs, nmc, L_)
    n_batch = coord_v.shape[0]
    n = coord_v.shape[1] // 3
    b_i = np.repeat(np.arange(n_batch, dtype=np.int64), n)

    # Flat element offsets (per core, local slab coordinates).
    flat_ones = ((b_i % BL) * SLAB + m_i.astype(np.int64) * PLANE
                 + y_i.astype(np.int64) * L_ + x_i.astype(np.int64))
    flat_z = flat_ones + HALF

    in_maps = []
    pts_per_core = BL * n  # 320
    for c in range(NCORES):
        sel = slice(c * pts_per_core, (c + 1) * pts_per_core)
        po = flat_ones[sel]
        pz = flat_z[sel]
        offs_np = np.zeros((128, N_SCATTER_COLS), dtype=np.int32)
        offs_np[:, 0] = po[0:128]
        offs_np[:, 1] = pz[0:128]
        offs_np[:, 2] = po[128:256]
        offs_np[:, 3] = pz[128:256]
        offs_np[0:64, 4] = po[256:320]
        offs_np[64:128, 4] = pz[256:320]
        in_maps.append({"offs": offs_np})
    return in_maps


def _run(in_maps, **kwargs):
    if "nc" not in _CACHE:
        _CACHE["nc"] = _build_nc()
    nc = _CACHE["nc"]
    from concourse.bass_utils import run_bass_kernel_spmd
    return run_bass_kernel_spmd(nc, in_maps, core_ids=list(range(NCORES)),
                                **kwargs)


def kernel(coord_v, lows, highs, nmc, L):
    nmc = int(nmc)
    L_ = int(L)
    assert nmc == NMC and L_ == globals()["L"], (nmc, L_)

    in_maps = _prepare_in_maps(coord_v, lows, highs, nmc, L_)
    res = _run(in_maps)
    parts = [res.results[c]["out"].reshape(BL, 2 * NMC, L_, L_)
             for c in range(NCORES)]
    return np.concatenate(parts, axis=0)


# revision 15
# speedup vs baseline: 1.0408x; 1.0408x over previous
"""Trainium2 Bass kernel for nn_CustomParameterTransform (scatter_memory).

Reference semantics: coord_v [256, 30] holds 10 (x, y, mass) triplets per
sample. Each triplet maps to integer grid indices (x_i, y_i, m_i); a one-hot
volume z [B, 16, 128, 128] is scattered (z[b, m, y, x] = 1) and the output is
concat(1-z, z) over the channel axis -> [256, 32, 128, 128] f32 (512 MB).

Strategy (8 NeuronCores, batch-sharded, no cross-core comm):
  - The output is almost entirely constant: the first 16 channels are 1.0
    except at scatter points, the last 16 are 0.0 except at scatter points.
  - Per core (32 samples, 64 MB slab): fill the slab from a constant SBUF
    tile with large DMAs (write-only HBM traffic, the memory roofline),
    then fix up the 640 scatter points with indirect (scatter) DMAs.
  - Indices are computed on the host with the exact same jax ops as the
    reference (bit-identical floor/log10 behavior) and passed per-core as a
    [128, 5] int32 tensor of flat element offsets.
"""

import numpy as np

B = 256
NSRC = 10
NMC = 16
L = 128
NCORES = 8
BL = B // NCORES          # 32 samples per core
PLANE = L * L             # 16384
HALF = NMC * PLANE        # 262144 elements per half-slab
SLAB = 2 * HALF           # 524288 elements per sample
OUT_ELEMS = BL * SLAB     # 16777216 per core (64 MB)

N_SCATTER_COLS = 5        # 640 scatter writes = 128 partitions x 5 columns

_CACHE = {}


def _build_nc():
    import concourse.bass as bass
    import concourse.tile as tile
    from concourse import bacc, mybir
    from concourse.tile_rust import add_dep_helper

    nc = bacc.Bacc("TRN2", target_bir_lowering=False, debug=False,
                   num_devices=NCORES)

    offs = nc.dram_tensor("offs", [128, N_SCATTER_COLS], mybir.dt.int32,
                          kind="ExternalInput").ap()
    out = nc.dram_tensor("out", [OUT_ELEMS], mybir.dt.float32,
                         kind="ExternalOutput").ap()

    with tile.TileContext(nc) as tc:
        with tc.tile_pool(name="src", bufs=1) as src_pool, \
             tc.tile_pool(name="small", bufs=1) as small_pool:
            # Constant source tiles, in increasing size so fills can start
            # as soon as the smallest is ready while the bigger ones memset
            # in the background:
            #  - minis ([128, 512], ~0.5 us memsets on gpsimd, the engine
            #    that consistently wakes first) feed slabs 0-1;
            #  - combo_t is one full 2 MB slab ([128, 4096]: DMA iterates
            #    partition-major, so partitions 0-63 are the ones half,
            #    64-127 the zeros half; ~3.5 us memsets) feeds slabs 2-9;
            #  - mega_t is two slabs ([128, 8192]: slab = 64 partitions, so
            #    ones iff p%64 < 32; ~7 us memsets, ready ~25 us) feeds
            #    slabs 10-31 as 4 MB fills — halving the ~390 ns HWDGE
            #    inter-fill ring gaps.
            ones_mini = src_pool.tile([128, 512], mybir.dt.float32)
            zeros_mini = src_pool.tile([128, 512], mybir.dt.float32)
            nc.gpsimd.memset(ones_mini[:, :], 1.0)
            nc.gpsimd.memset(zeros_mini[:, :], 0.0)
            combo_t = src_pool.tile([128, 4096], mybir.dt.float32)
            nc.vector.memset(combo_t[0:64, :], 1.0)
            nc.gpsimd.memset(combo_t[64:128, :], 0.0)
            mega_t = src_pool.tile([128, 8192], mybir.dt.float32)
            nc.vector.memset(mega_t[0:32, :], 1.0)
            nc.gpsimd.memset(mega_t[32:64, :], 0.0)
            nc.vector.memset(mega_t[64:96, :], 1.0)
            nc.gpsimd.memset(mega_t[96:128, :], 0.0)

            # Scatter offsets: [128, 5] int32 flat element indices.
            # Column layout (entries are points p = 10*s + k, in order):
            #   col 0: ones-half offsets for points   0..127 (samples  0-12)
            #   col 1: z-half    offsets for points   0..127 (samples  0-12)
            #   col 2: ones-half offsets for points 128..255 (samples 12-25)
            #   col 3: z-half    offsets for points 128..255 (samples 12-25)
            #   col 4: rows 0-63 ones-half pts 256..319, rows 64-127 z-half
            #          pts 256..319 (samples 25-31)
            # offs load and vals memsets are only needed by the scatters
            # (earliest ~70 us in) — keep them off the fill rings / after
            # the mini-tile memsets so they don't delay the first fills.
            offs_t = small_pool.tile([128, N_SCATTER_COLS], mybir.dt.int32)
            nc.gpsimd.dma_start(offs_t[:, :], offs[:, :])
            vals_t = small_pool.tile([128, N_SCATTER_COLS], mybir.dt.float32)
            nc.gpsimd.memset(vals_t[:, 0:1], 0.0)
            nc.gpsimd.memset(vals_t[:, 1:2], 1.0)
            nc.gpsimd.memset(vals_t[:, 2:3], 0.0)
            nc.gpsimd.memset(vals_t[:, 3:4], 1.0)
            nc.gpsimd.memset(vals_t[0:64, 4:5], 0.0)
            nc.gpsimd.memset(vals_t[64:128, 4:5], 1.0)

            MINI = 65536  # elements per mini fill (256 KB)
            ones_fills = {}   # sample -> list of fills covering its ones half
            zeros_fills = {}  # sample -> list of fills covering its zeros half
            for s in (0, 1):
                e_ones = nc.sync if s == 0 else nc.scalar
                e_zeros = nc.scalar if s == 0 else nc.sync
                ones_fills[s] = [
                    e_ones.dma_start(
                        out[s * SLAB + k * MINI:s * SLAB + (k + 1) * MINI],
                        ones_mini[:, :])
                    for k in range(4)]
                zeros_fills[s] = [
                    e_zeros.dma_start(
                        out[s * SLAB + HALF + k * MINI:
                            s * SLAB + HALF + (k + 1) * MINI],
                        zeros_mini[:, :])
                    for k in range(4)]
            for s in range(2, 10):
                eng = nc.sync if s % 2 == 0 else nc.scalar
                f = eng.dma_start(out[s * SLAB:(s + 1) * SLAB], combo_t[:, :])
                ones_fills[s] = [f]
                zeros_fills[s] = [f]
            for s in range(10, BL, 2):
                eng = nc.sync if (s // 2) % 2 == 0 else nc.scalar
                f = eng.dma_start(out[s * SLAB:(s + 2) * SLAB], mega_t[:, :])
                for ss in (s, s + 1):
                    ones_fills[ss] = [f]
                    zeros_fills[ss] = [f]

            # Which sample-fills each scatter column touches.
            def deps(table, lo, hi):
                return [f for s in range(lo, hi) for f in table[s]]
            col_deps = [
                deps(ones_fills, 0, 13),
                deps(zeros_fills, 0, 13),
                deps(ones_fills, 12, 26),
                deps(zeros_fills, 12, 26),
                deps(ones_fills, 25, BL) + deps(zeros_fills, 25, BL),
            ]

            # Narrow declared out AP ([1, 1] at offset 0): the real write
            # addresses come from the offset tensor; a full-tensor AP would
            # make Tile serialize every scatter behind every fill (WAW), and
            # the explicit col_deps edges below provide the true ordering.
            out2d = out[0:1].unsqueeze(1)
            for j in range(N_SCATTER_COLS):
                sc = nc.gpsimd.indirect_dma_start(
                    out=out2d,
                    out_offset=bass.IndirectOffsetOnAxis(
                        ap=offs_t[:, j:j + 1], axis=0),
                    in_=vals_t[:, j:j + 1],
                    in_offset=None,
                )
                for fl in col_deps[j]:
                    add_dep_helper(sc.ins, fl.ins,
                                   reason="scatter after its sample fills")

    nc.compile()
    return nc


def _compute_indices(coord_v, lows, highs, nmc, L_):
    """Replicates reference.py lines exactly (same jax ops on the default
    device) so the floor/log10 bin boundaries match bit-for-bit."""
    import jax.numpy as jnp

    cv = jnp.asarray(np.asarray(coord_v, dtype=np.float32))
    n = cv.shape[1] // 3
    v10 = cv.at[:, 2::3].set(jnp.log10(cv[:, 2::3]))
    lo = jnp.tile(jnp.asarray(np.asarray(lows, dtype=np.float32)), n)
    hi = jnp.tile(jnp.asarray(np.asarray(highs, dtype=np.float32)), n)
    coord_grid = (v10 - lo) / (hi - lo)
    tr = coord_grid.reshape(-1, 3)
    x_i = jnp.floor(tr[:, 0] * L_).astype(jnp.int32)
    y_i = jnp.floor(tr[:, 1] * L_).astype(jnp.int32)
    m_i = jnp.floor(tr[:, 2] * nmc).astype(jnp.int32)
    return (np.asarray(x_i), np.asarray(y_i), np.asarray(m_i))


def _prepare_in_maps(coord_v, lows, highs, nmc, L):
    nmc = int(nmc)
    L_ = int(L)
    x_i, y_i, m_i = _compute_indices(coord_v, lows, highs, nmc, L_)
    n_batch = coord_v.shape[0]
    n = coord_v.shape[1] // 3
    b_i = np.repeat(np.arange(n_batch, dtype=np.int64), n)

    # Flat element offsets (per core, local slab coordinates).
    flat_ones = ((b_i % BL) * SLAB + m_i.astype(np.int64) * PLANE
                 + y_i.astype(np.int64) * L_ + x_i.astype(np.int64))
    flat_z = flat_ones + HALF

    in_maps = []
    pts_per_core = BL * n  # 320
    for c in range(NCORES):
        sel = slice(c * pts_per_core, (c + 1) * pts_per_core)
        po = flat_ones[sel]
        pz = flat_z[sel]
        offs_np = np.zeros((128, N_SCATTER_COLS), dtype=np.int32)
        offs_np[:, 0] = po[0:128]
        offs_np[:, 1] = pz[0:128]
        offs_np[:, 2] = po[128:256]
        offs_np[:, 3] = pz[128:256]
        offs_np[0:64, 4] = po[256:320]
        offs_np[64:128, 4] = pz[256:320]
        in_maps.append({"offs": offs_np})
    return in_maps


def _run(in_maps, **kwargs):
    if "nc" not in _CACHE:
        _CACHE["nc"] = _build_nc()
    nc = _CACHE["nc"]
    from concourse.bass_utils import run_bass_kernel_spmd
    return run_bass_kernel_spmd(nc, in_maps, core_ids=list(range(NCORES)),
                                **kwargs)


def kernel(coord_v, lows, highs, nmc, L):
    nmc = int(nmc)
    L_ = int(L)
    assert nmc == NMC and L_ == globals()["L"], (nmc, L_)

    in_maps = _prepare_in_maps(coord_v, lows, highs, nmc, L_)
    res = _run(in_maps)
    parts = [res.results[c]["out"].reshape(BL, 2 * NMC, L_, L_)
             for c in range(NCORES)]
    return np.concatenate(parts, axis=0)
